# revision 13
# baseline (speedup 1.0000x reference)
"""Trainium2 Bass kernel for JonbertaSelfAttention (B=4,S=1024,DM=1024,H=16,D=64,SE=512,DF=512).

Sharding: 8 cores = (batch b = c//2) x (query-half qh = c%2), ONE NEFF for all
cores (query-half offset folded into per-core data: shifted distance tables and
a pre-sliced hidden_states column block).

v3: fp8e4 DoubleRow everywhere + single merged band/attention pipeline.
 - projections contract it-pairs [128,2,*];
 - QK^T scores contract d=64 via a zero-slot second lane (kT4/qT4/fkT4 carry a
   zeroed e=1 lane, rhs broadcast stride-0);
 - relative-position bias via banded matmuls (x16-scaled fp8 tables), DRAM
   skew round-trip (SWDGE queue), then paired-identity DoubleRow
   transpose/inject (identsplit/identz carry 1/16 to undo the table scale);
 - PV produces ctx in [l, d+1] layout (lhsT=exp pairs) so softmax denominators
   are per-partition scalars; normalized ctx is transposed back by PE;
 - output dense contracts head-pairs from fp8 ctxpk;
 - band evacuation (PSUM->SBUF) split between ACT and DVE; LN applies on Pool.
"""
import os
import numpy as np
import ml_dtypes

BF16 = ml_dtypes.bfloat16
F8 = ml_dtypes.float8_e4m3
B, S, DM, H, D, SE, DF, MAXP = 4, 1024, 1024, 16, 64, 512, 512, 1024
L = 512          # query rows per core
NRT = S // 128   # 8 r-tiles
NLT = L // 128   # 4 l-tiles
NET = SE // 128  # 4 encoder r-tiles
LN_EPS = 1e-12

_CACHE = {}
LAST_EXEC_NS = None
LAST_MEAN_EXEC_NS = None
LAST_TRACE = None

# band-evacuation engine split (True = ACT, False = DVE), tuned vs trace
Q_ON_ACT = (True, False, True, False)
K_ON_ACT = (True, False, False, False, True, False, False, False)


def _build():
    import concourse.bass as bass
    import concourse.mybir as mybir
    import concourse.tile as tile
    from concourse import bacc
    from concourse.masks import make_identity
    from contextlib import ExitStack

    dt = mybir.dt
    nc = bacc.Bacc("TRN2", target_bir_lowering=False, debug=False, num_devices=8)

    d_hsT = nc.dram_tensor("hsT", [DM, S], dt.float8e4, kind="ExternalInput")
    d_hsTq = nc.dram_tensor("hsTq", [DM, L], dt.float8e4, kind="ExternalInput")
    d_hsres = nc.dram_tensor("hsres", [L, DM], dt.bfloat16, kind="ExternalInput")
    d_encT = nc.dram_tensor("encT", [DF, SE], dt.float8e4, kind="ExternalInput")
    d_mask = nc.dram_tensor("mask", [S], dt.float32, kind="ExternalInput")
    d_wqT = nc.dram_tensor("wqT", [DM, DM], dt.float8e4, kind="ExternalInput")
    d_wkT = nc.dram_tensor("wkT", [DM, DM], dt.float8e4, kind="ExternalInput")
    d_wvT = nc.dram_tensor("wvT", [DM, DM], dt.float8e4, kind="ExternalInput")
    d_wfkT = nc.dram_tensor("wfkT", [DF, DM], dt.float8e4, kind="ExternalInput")
    d_wfvT = nc.dram_tensor("wfvT", [DF, DM], dt.float8e4, kind="ExternalInput")
    d_woT = nc.dram_tensor("woT", [DM, DM], dt.float8e4, kind="ExternalInput")
    d_bq = nc.dram_tensor("bq", [DM], dt.float32, kind="ExternalInput")
    d_bk = nc.dram_tensor("bk", [DM], dt.float32, kind="ExternalInput")
    d_bfk = nc.dram_tensor("bfk", [DM], dt.float32, kind="ExternalInput")
    d_bv = nc.dram_tensor("bv", [DM], dt.bfloat16, kind="ExternalInput")
    d_bfv = nc.dram_tensor("bfv", [DM], dt.bfloat16, kind="ExternalInput")
    d_lng = nc.dram_tensor("lng", [DM], dt.float32, kind="ExternalInput")
    d_lnb = nc.dram_tensor("lnb", [DM], dt.float32, kind="ExternalInput")
    d_distn = nc.dram_tensor("distn", [D, 2048], dt.float8e4, kind="ExternalInput")
    d_distf = nc.dram_tensor("distf", [D, 2048], dt.float8e4, kind="ExternalInput")
    d_out = nc.dram_tensor("out", [L, DM], dt.float32, kind="ExternalOutput")

    AP = bass.AP
    f32 = dt.float32
    bf16 = dt.bfloat16
    fp8 = dt.float8e4
    AF = mybir.ActivationFunctionType
    DR = mybir.MatmulPerfMode.DoubleRow
    ALU = mybir.AluOpType

    with tile.TileContext(nc) as tc, ExitStack() as top:
        scr = top.enter_context(tc.tile_pool(name="scr", bufs=H, space="DRAM"))
        scr2 = top.enter_context(tc.tile_pool(name="scr2", bufs=H, space="DRAM"))
        pers = top.enter_context(tc.tile_pool(name="pers", bufs=1))
        # e=1 lanes of kT4/qT4/fkT4 are zeros (DoubleRow zero-slot padding)
        kT4 = pers.tile([128, NRT, 2, S], fp8, tag="kT4")
        qT4 = pers.tile([128, NRT, 2, L], fp8, tag="qT4")
        fkT4 = pers.tile([128, NRT, 2, SE], fp8, tag="fkT4")
        v_sb = pers.tile([128, NRT, H, 65], fp8, tag="v_sb")
        fv_sb = pers.tile([128, NET, H, 65], fp8, tag="fv_sb")
        hsres = pers.tile([128, NLT, DM], bf16, tag="hsres")
        ctxpk = pers.tile([64, NRT, 2, L], fp8, tag="ctxpk")
        bv_b = pers.tile([128, DM], bf16, tag="bv_b")
        bfv_b = pers.tile([128, DM], bf16, tag="bfv_b")
        lng_b = pers.tile([128, DM], f32, tag="lng_b")
        lnb_b = pers.tile([128, DM], f32, tag="lnb_b")
        bq_s = pers.tile([128, NRT], f32, tag="bq_s")
        bk_s = pers.tile([128, NRT], f32, tag="bk_s")
        bfk_s = pers.tile([128, NRT], f32, tag="bfk_s")
        msk = pers.tile([128, NRT], f32, tag="msk")
        identb = pers.tile([128, 128], bf16, tag="identb")
        identsplit = pers.tile([128, 2, 256], fp8, tag="identsplit")
        identz = pers.tile([128, 2, 128], fp8, tag="identz")
        eps_t = pers.tile([128, 1], f32, tag="eps_t")
        zero_t = pers.tile([128, 1], f32, tag="zero_t")

        make_identity(nc, identb[:])
        nc.vector.memset(eps_t[:], LN_EPS)
        nc.vector.memset(zero_t[:], 0.0)
        # zero-slot lanes + identity patterns (big memsets on idle Pool engine)
        nc.gpsimd.memset(kT4[:, :, 1, :], 0.0)
        nc.gpsimd.memset(qT4[:, :, 1, :], 0.0)
        nc.gpsimd.memset(fkT4[:, :, 1, :], 0.0)
        nc.gpsimd.memset(identsplit[:], 0.0)
        nc.gpsimd.memset(identz[:, 1, :], 0.0)
        # identsplit: e=0 -> I/16 in cols 0:128, e=1 -> I/16 in cols 128:256
        nc.scalar.activation(out=identsplit[:, 0, 0:128], in_=identb[:],
                             func=AF.Identity, scale=0.0625)
        nc.scalar.activation(out=identsplit[:, 1, 128:256], in_=identb[:],
                             func=AF.Identity, scale=0.0625)
        nc.scalar.activation(out=identz[:, 0, :], in_=identb[:],
                             func=AF.Identity, scale=0.0625)
        nc.sync.dma_start(out=bq_s[:], in_=AP(tensor=d_bq, offset=0, ap=[[1, 128], [128, NRT]]))
        nc.sync.dma_start(out=bk_s[:], in_=AP(tensor=d_bk, offset=0, ap=[[1, 128], [128, NRT]]))
        nc.sync.dma_start(out=bfk_s[:], in_=AP(tensor=d_bfk, offset=0, ap=[[1, 128], [128, NRT]]))
        nc.sync.dma_start(out=msk[:], in_=AP(tensor=d_mask, offset=0, ap=[[1, 128], [128, NRT]]))
        nc.vector.memset(v_sb[:, :, :, 64:65], 1.0)
        nc.vector.memset(fv_sb[:, :, :, 64:65], 1.0)

        pb = top.enter_context(tc.tile_pool(name="pb", bufs=1))
        hsT = pb.tile([128, NRT, S], fp8, tag="hsT")
        hsTq = pb.tile([128, NRT, L], fp8, tag="hsTq")
        encT = pb.tile([128, NET, SE], fp8, tag="encT")
        distn_s = pb.tile([128, 2048], fp8, tag="distn")
        distf_s = pb.tile([128, 2048], fp8, tag="distf")

        nc.sync.dma_start(out=hsTq[:], in_=AP(tensor=d_hsTq, offset=0,
                                              ap=[[L, 128], [128 * L, NRT], [1, L]]))
        nc.sync.dma_start(out=hsT[:], in_=AP(tensor=d_hsT, offset=0,
                                             ap=[[S, 128], [128 * S, NRT], [1, S]]))
        nc.sync.dma_start(out=encT[:], in_=AP(tensor=d_encT, offset=0,
                                              ap=[[SE, 128], [128 * SE, NET], [1, SE]]))
        for half in range(2):
            nc.sync.dma_start(out=distn_s[half * 64:(half + 1) * 64, :],
                              in_=AP(tensor=d_distn, offset=0, ap=[[2048, 64], [1, 2048]]))
            nc.sync.dma_start(out=distf_s[half * 64:(half + 1) * 64, :],
                              in_=AP(tensor=d_distf, offset=0, ap=[[2048, 64], [1, 2048]]))

        cq_dram = {}
        ck_dram = {}

        # ---------- phase B1: all dense projections ----------
        with ExitStack() as phB:
            wp = phB.enter_context(tc.tile_pool(name="wp", bufs=1))
            wq_all = wp.tile([128, NRT, DM], fp8, tag="wq_all")
            wk_all = wp.tile([128, NRT, DM], fp8, tag="wk_all")
            wfk_all = wp.tile([128, NET, DM], fp8, tag="wfk_all")
            wv_k = wp.tile([128, NRT, DM], fp8, tag="wv_k")
            wfv_k = wp.tile([128, NET, DM], fp8, tag="wfv_k")
            pp_proj = phB.enter_context(tc.tile_pool(name="pp_proj", bufs=2, space="PSUM"))

            nc.sync.dma_start(out=wq_all[:], in_=AP(tensor=d_wqT, offset=0,
                                                    ap=[[DM, 128], [128 * DM, NRT], [1, DM]]))
            nc.sync.dma_start(out=wk_all[:], in_=AP(tensor=d_wkT, offset=0,
                                                    ap=[[DM, 128], [128 * DM, NRT], [1, DM]]))
            nc.sync.dma_start(out=wfk_all[:], in_=AP(tensor=d_wfkT, offset=0,
                                                     ap=[[DM, 128], [128 * DM, NET], [1, DM]]))
            nc.sync.dma_start(out=wv_k[:], in_=AP(tensor=d_wvT, offset=0,
                                                  ap=[[DM, 128], [128 * DM, NRT], [1, DM]]))
            nc.sync.dma_start(out=wfv_k[:], in_=AP(tensor=d_wfvT, offset=0,
                                                   ap=[[DM, 128], [128 * DM, NET], [1, DM]]))
            nc.sync.dma_start(out=bv_b[:], in_=AP(tensor=d_bv, offset=0, ap=[[0, 128], [1, DM]]))
            nc.sync.dma_start(out=bfv_b[:], in_=AP(tensor=d_bfv, offset=0, ap=[[0, 128], [1, DM]]))

            for ot in range(NRT):
                # Q projection (local query half)
                ps = pp_proj.tile([128, 512], f32, tag="proj")
                for j in range(NRT // 2):
                    nc.tensor.matmul(ps[:], lhsT=wq_all[:, 2 * j:2 * j + 2, ot * 128:(ot + 1) * 128],
                                     rhs=hsTq[:, 2 * j:2 * j + 2, :],
                                     start=(j == 0), stop=(j == NRT // 2 - 1), perf_mode=DR)
                nc.scalar.activation(out=qT4[:, ot, 0, :], in_=ps[:], func=AF.Identity,
                                     bias=bq_s[:, ot:ot + 1], scale=1.0)
                # K projection (full sequence)
                for sb_i in range(2):
                    ps = pp_proj.tile([128, 512], f32, tag="proj")
                    for j in range(NRT // 2):
                        nc.tensor.matmul(ps[:], lhsT=wk_all[:, 2 * j:2 * j + 2, ot * 128:(ot + 1) * 128],
                                         rhs=hsT[:, 2 * j:2 * j + 2, sb_i * 512:(sb_i + 1) * 512],
                                         start=(j == 0), stop=(j == NRT // 2 - 1), perf_mode=DR)
                    nc.scalar.activation(out=kT4[:, ot, 0, sb_i * 512:(sb_i + 1) * 512], in_=ps[:],
                                         func=AF.Identity, bias=bk_s[:, ot:ot + 1], scale=1.0)
                # FK projection
                ps = pp_proj.tile([128, 512], f32, tag="proj")
                for j in range(NET // 2):
                    nc.tensor.matmul(ps[:], lhsT=wfk_all[:, 2 * j:2 * j + 2, ot * 128:(ot + 1) * 128],
                                     rhs=encT[:, 2 * j:2 * j + 2, :],
                                     start=(j == 0), stop=(j == NET // 2 - 1), perf_mode=DR)
                nc.scalar.activation(out=fkT4[:, ot, 0, :], in_=ps[:], func=AF.Identity,
                                     bias=bfk_s[:, ot:ot + 1], scale=1.0)
            # V / FV projections (s-major)
            for st in range(NRT):
                for ob in range(2):
                    ps = pp_proj.tile([128, 512], f32, tag="proj")
                    for j in range(NRT // 2):
                        nc.tensor.matmul(ps[:], lhsT=hsT[:, 2 * j:2 * j + 2, st * 128:(st + 1) * 128],
                                         rhs=wv_k[:, 2 * j:2 * j + 2, ob * 512:(ob + 1) * 512],
                                         start=(j == 0), stop=(j == NRT // 2 - 1), perf_mode=DR)
                    nc.vector.tensor_add(
                        out=v_sb[:, st, ob * 8:(ob + 1) * 8, 0:64],
                        in0=ps[:].rearrange("p (h d) -> p h d", d=64),
                        in1=bv_b[:, ob * 512:(ob + 1) * 512].rearrange("p (h d) -> p h d", d=64))
            for st in range(NET):
                for ob in range(2):
                    ps = pp_proj.tile([128, 512], f32, tag="proj")
                    for j in range(NET // 2):
                        nc.tensor.matmul(ps[:], lhsT=encT[:, 2 * j:2 * j + 2, st * 128:(st + 1) * 128],
                                         rhs=wfv_k[:, 2 * j:2 * j + 2, ob * 512:(ob + 1) * 512],
                                         start=(j == 0), stop=(j == NET // 2 - 1), perf_mode=DR)
                    nc.vector.tensor_add(
                        out=fv_sb[:, st, ob * 8:(ob + 1) * 8, 0:64],
                        in0=ps[:].rearrange("p (h d) -> p h d", d=64),
                        in1=bfv_b[:, ob * 512:(ob + 1) * 512].rearrange("p (h d) -> p h d", d=64))

        # ---------- merged bands + attention pipeline ----------
        with ExitStack() as phC:
            bsb = phC.enter_context(tc.tile_pool(name="bsb", bufs=2))
            bsb2 = phC.enter_context(tc.tile_pool(name="bsb2", bufs=2))
            gp = phC.enter_context(tc.tile_pool(name="gp", bufs=2))
            g2 = phC.enter_context(tc.tile_pool(name="g2", bufs=2))
            ep = phC.enter_context(tc.tile_pool(name="ep", bufs=6))
            cp = phC.enter_context(tc.tile_pool(name="cp", bufs=4))
            rp = phC.enter_context(tc.tile_pool(name="rp", bufs=4))
            pp_band = phC.enter_context(tc.tile_pool(name="pp_band", bufs=1, space="PSUM"))
            pp_s = phC.enter_context(tc.tile_pool(name="pp_s", bufs=2, space="PSUM"))
            pp_c = phC.enter_context(tc.tile_pool(name="pp_c", bufs=1, space="PSUM"))

            def emit_bands(h):
                hp = (h % 2) * 64
                ot = h // 2
                cq = scr.tile([NLT * 128, 1152], fp8, tag="cq")
                cq_dram[h] = cq
                qstage = bsb.tile([128, NLT, 1152], fp8, tag="qstage")
                for lt in range(NLT):
                    bm = 896 - lt * 128
                    ps = pp_band.tile([128, 1152], f32, tag="band")
                    lhs = qT4[hp:hp + 64, ot, :, lt * 128:(lt + 1) * 128]
                    for n0, nn in ((0, 512), (512, 512), (1024, 128)):
                        rhs = distf_s[hp:hp + 64, bm + n0:bm + n0 + nn] \
                            .unsqueeze(1).broadcast_to([64, 2, nn])
                        nc.tensor.matmul(ps[:, n0:n0 + nn], lhsT=lhs, rhs=rhs,
                                         start=True, stop=True, perf_mode=DR)
                    if Q_ON_ACT[lt]:
                        nc.scalar.copy(out=qstage[:, lt, :], in_=ps[:])
                    else:
                        nc.vector.tensor_copy(out=qstage[:, lt, :], in_=ps[:])
                nc.gpsimd.dma_start(out=AP(tensor=cq.tensor, offset=cq.offset,
                                           ap=[[1152, 128], [128 * 1152, NLT], [1, 1152]]),
                                    in_=qstage[:])
                ck = scr2.tile([NRT * 128, 640], fp8, tag="ck")
                ck_dram[h] = ck
                kstage = bsb2.tile([128, NRT, 640], fp8, tag="kstage")
                for rt in range(NRT):
                    bt = 896 - 128 * rt
                    ps = pp_band.tile([128, 1152], f32, tag="band")
                    lhs = kT4[hp:hp + 64, ot, :, rt * 128:(rt + 1) * 128]
                    for n0, nn in ((0, 512), (512, 128)):
                        rhs = distn_s[hp:hp + 64, bt + n0:bt + n0 + nn] \
                            .unsqueeze(1).broadcast_to([64, 2, nn])
                        nc.tensor.matmul(ps[:, n0:n0 + nn], lhsT=lhs, rhs=rhs,
                                         start=True, stop=True, perf_mode=DR)
                    if K_ON_ACT[rt]:
                        nc.scalar.activation(out=kstage[:, rt, :], in_=ps[:, 0:640],
                                             func=AF.Identity, bias=msk[:, rt:rt + 1],
                                             scale=1.0)
                    else:
                        nc.vector.scalar_tensor_tensor(
                            out=kstage[:, rt, :], in0=ps[:, 0:640], scalar=msk[:, rt:rt + 1],
                            in1=hsT[:, 0, 0:640], op0=ALU.add, op1=ALU.bypass)
                nc.gpsimd.dma_start(out=AP(tensor=ck.tensor, offset=ck.offset,
                                           ap=[[640, 128], [128 * 640, NRT], [1, 640]]),
                                    in_=kstage[:])

            def attention(h):
                hp = (h % 2) * 64
                ot = h // 2
                b1all = gp.tile([128, NLT, 1024], fp8, tag="b1all")
                src = cq_dram[h]
                nc.gpsimd.dma_start(out=b1all[:], in_=AP(
                    tensor=src.tensor, offset=src.offset + 127,
                    ap=[[1151, 128], [128 * 1152, NLT], [1, 1024]]))
                b2all = g2.tile([128, NRT, 512], fp8, tag="b2all")
                src = ck_dram[h]
                nc.gpsimd.dma_start(out=b2all[:], in_=AP(
                    tensor=src.tensor, offset=src.offset + 127,
                    ap=[[639, 128], [128 * 640, NRT], [1, 512]]))
                psA = pp_c.tile([128, NLT, 65], f32, tag="ctxA")
                psB = pp_c.tile([128, NLT, 65], f32, tag="ctxB")
                ctxT_ps = pp_c.tile([64, 512], f32, tag="ctxT")

                def scores_rt(rt):
                    ps = pp_s.tile([128, 512], f32, tag="sc")
                    nc.tensor.matmul(
                        ps[:],
                        lhsT=kT4[hp:hp + 64, ot, :, rt * 128:(rt + 1) * 128],
                        rhs=qT4[hp:hp + 64, ot, 0, :].unsqueeze(1).broadcast_to([64, 2, L]),
                        start=True, stop=False, perf_mode=DR, skip_group_check=True)
                    for ltp in (0, 2):
                        nc.tensor.matmul(
                            ps[:, ltp * 128:ltp * 128 + 256],
                            lhsT=b1all[:, ltp:ltp + 2, rt * 128:(rt + 1) * 128],
                            rhs=identsplit[:], start=False, stop=False,
                            perf_mode=DR, skip_group_check=True)
                    nc.tensor.matmul(
                        ps[:], lhsT=identz[:],
                        rhs=b2all[:, rt, :].unsqueeze(1).broadcast_to([128, 2, 512]),
                        start=False, stop=True, perf_mode=DR, skip_group_check=True)
                    return ps

                # scores/exp stream first (start=True clears the whole PSUM
                # bank's has_written bits, so PV must run lb-outer afterwards)
                exs = {}
                for u in range(4):
                    ex4 = ep.tile([128, 2, 512], fp8, tag="ex")
                    for half in range(2):
                        ps = scores_rt(2 * u + half)
                        nc.scalar.activation(out=ex4[:, half, :], in_=ps[:], func=AF.Exp,
                                             bias=zero_t[:], scale=0.125)
                    exs[u] = ex4
                for lb in range(NLT):
                    for u in range(4):
                        nc.tensor.matmul(
                            psA[:, lb, :], lhsT=exs[u][:, :, lb * 128:(lb + 1) * 128],
                            rhs=v_sb[:, 2 * u:2 * u + 2, h, :],
                            start=(u == 0), stop=(u == 3),
                            perf_mode=DR, skip_group_check=True)

                # encoder branch
                eexs = {}
                for p in range(2):
                    eex4 = ep.tile([128, 2, 512], fp8, tag="ex")
                    for half in range(2):
                        ret = 2 * p + half
                        ps = pp_s.tile([128, 512], f32, tag="sc")
                        nc.tensor.matmul(
                            ps[:],
                            lhsT=fkT4[hp:hp + 64, ot, :, ret * 128:(ret + 1) * 128],
                            rhs=qT4[hp:hp + 64, ot, 0, :].unsqueeze(1).broadcast_to([64, 2, L]),
                            start=True, stop=True, perf_mode=DR, skip_group_check=True)
                        nc.scalar.activation(out=eex4[:, half, :], in_=ps[:], func=AF.Exp,
                                             bias=zero_t[:], scale=0.125)
                    eexs[p] = eex4
                for lb in range(NLT):
                    for p in range(2):
                        nc.tensor.matmul(
                            psB[:, lb, :], lhsT=eexs[p][:, :, lb * 128:(lb + 1) * 128],
                            rhs=fv_sb[:, 2 * p:2 * p + 2, h, :],
                            start=(p == 0), stop=(p == 1),
                            perf_mode=DR, skip_group_check=True)

                # normalize per l-partition (denominators are per-partition scalars)
                recA = rp.tile([128, NLT, 1], f32, tag="recA")
                recB = rp.tile([128, NLT, 1], f32, tag="recB")
                nc.vector.reciprocal(out=recA[:], in_=psA[:, :, 64:65])
                nc.vector.reciprocal(out=recB[:], in_=psB[:, :, 64:65])
                tnf = cp.tile([128, NLT, 64], bf16, tag="tnf")
                for lb in range(NLT):
                    t1 = cp.tile([128, 64], f32, tag="t1")
                    nc.vector.tensor_scalar(out=t1[:], in0=psA[:, lb, 0:64],
                                            scalar1=recA[:, lb, :], scalar2=None,
                                            op0=ALU.mult)
                    nc.vector.scalar_tensor_tensor(
                        out=tnf[:, lb, :], in0=psB[:, lb, 0:64], scalar=recB[:, lb, :],
                        in1=t1[:], op0=ALU.mult, op1=ALU.add)
                # transpose normalized ctx back to [d, l] and stash as fp8
                for lb in range(NLT):
                    nc.tensor.matmul(ctxT_ps[:, lb * 128:(lb + 1) * 128],
                                     lhsT=tnf[:, lb, :], rhs=identb[:],
                                     start=True, stop=True, skip_group_check=True)
                nc.vector.tensor_copy(out=ctxpk[:, ot, h % 2, :], in_=ctxT_ps[:])

            for ot in range(NRT):
                emit_bands(2 * ot)
                emit_bands(2 * ot + 1)
                if ot >= 1:
                    attention(2 * (ot - 1))
                    attention(2 * (ot - 1) + 1)
            attention(2 * (NRT - 1))
            attention(2 * (NRT - 1) + 1)

        # ---------- output dense + residual + LN ----------
        nc.sync.dma_start(out=lng_b[:], in_=AP(tensor=d_lng, offset=0, ap=[[0, 128], [1, DM]]))
        nc.sync.dma_start(out=lnb_b[:], in_=AP(tensor=d_lnb, offset=0, ap=[[0, 128], [1, DM]]))
        nc.sync.dma_start(out=hsres[:], in_=AP(tensor=d_hsres, offset=0,
                                               ap=[[DM, 128], [128 * DM, NLT], [1, DM]]))

        with ExitStack() as phD:
            pd = phD.enter_context(tc.tile_pool(name="pd", bufs=1))
            wo_sb = pd.tile([64, H, DM], fp8, tag="wo_sb")
            yp = phD.enter_context(tc.tile_pool(name="yp", bufs=2))
            op = phD.enter_context(tc.tile_pool(name="op", bufs=2))
            stp = phD.enter_context(tc.tile_pool(name="stp", bufs=2))
            pp_y = phD.enter_context(tc.tile_pool(name="pp_y", bufs=2, space="PSUM"))

            nc.sync.dma_start(out=wo_sb[:], in_=AP(tensor=d_woT, offset=0,
                                                   ap=[[DM, 64], [64 * DM, H], [1, DM]]))
            for st in range(NLT):
                y = yp.tile([128, DM], f32, tag="y")
                for ob in range(2):
                    ps = pp_y.tile([128, 512], f32, tag="py")
                    for hq in range(NRT):
                        nc.tensor.matmul(
                            ps[:], lhsT=ctxpk[:, hq, :, st * 128:(st + 1) * 128],
                            rhs=wo_sb[:, 2 * hq:2 * hq + 2, ob * 512:(ob + 1) * 512],
                            start=(hq == 0), stop=(hq == NRT - 1), perf_mode=DR)
                    nc.vector.tensor_add(out=y[:, ob * 512:(ob + 1) * 512], in0=ps[:],
                                         in1=hsres[:, st, ob * 512:(ob + 1) * 512])
                stats = stp.tile([128, 2, 6], f32, tag="stats")
                nc.vector.bn_stats(out=stats[:, 0, :], in_=y[:, 0:512])
                nc.vector.bn_stats(out=stats[:, 1, :], in_=y[:, 512:1024])
                mv = stp.tile([128, 2], f32, tag="mv")
                nc.vector.bn_aggr(out=mv[:], in_=stats[:])
                sd = stp.tile([128, 1], f32, tag="sd")
                nc.scalar.activation(out=sd[:], in_=mv[:, 1:2], func=AF.Sqrt,
                                     bias=eps_t[:], scale=1.0)
                rstd = stp.tile([128, 1], f32, tag="rstd")
                nc.vector.reciprocal(out=rstd[:], in_=sd[:])
                o1 = op.tile([128, DM], f32, tag="o1")
                nc.vector.tensor_scalar(out=o1[:], in0=y[:], scalar1=mv[:, 0:1], scalar2=rstd[:],
                                        op0=ALU.subtract, op1=ALU.mult)
                o2 = op.tile([128, DM], f32, tag="o2")
                nc.gpsimd.tensor_mul(out=o2[:], in0=o1[:], in1=lng_b[:])
                o3 = op.tile([128, DM], f32, tag="o3")
                nc.gpsimd.tensor_add(out=o3[:], in0=o2[:], in1=lnb_b[:])
                nc.sync.dma_start(out=d_out[st * 128:(st + 1) * 128, :], in_=o3[:])

    nc.finalize()
    return nc


def _get_nc():
    if "nc" not in _CACHE:
        _CACHE["nc"] = _build()
    return _CACHE["nc"]


def kernel(**inputs):
    global LAST_EXEC_NS, LAST_MEAN_EXEC_NS, LAST_TRACE
    from concourse.bass_utils import run_bass_kernel_spmd

    inp = {k: np.asarray(v) for k, v in inputs.items()}
    hs = inp["hidden_states"].astype(np.float32)
    mask = inp["attention_mask"].astype(np.float32)
    enc = inp["encoder_hidden_states"].astype(np.float32)
    G = inp["dist_emb"].astype(np.float32)

    def b16(x):
        return np.ascontiguousarray(x.astype(BF16))

    def f8(x):
        return np.ascontiguousarray(x.astype(F8))

    shared = {
        "wqT": f8(inp["Wq"].T), "wkT": f8(inp["Wk"].T), "wvT": f8(inp["Wv"].T),
        "wfkT": f8(inp["Wfk"].T), "wfvT": f8(inp["Wfv"].T), "woT": f8(inp["Wo"].T),
        "bq": inp["bq"].astype(np.float32), "bk": inp["bk"].astype(np.float32),
        "bfk": inp["bfk"].astype(np.float32), "bv": b16(inp["bv"]), "bfv": b16(inp["bfv"]),
        "lng": inp["ln_g"].astype(np.float32), "lnb": inp["ln_b"].astype(np.float32),
    }
    # Padded tables: G' (natural order) and F' (flipped), plus per-query-half
    # shifted variants so one NEFF (band offsets hardcoded for l0=0) serves
    # both query halves. Tables ship x16 in fp8; the identity injectors carry
    # the 1/16 to undo it.
    Gp = np.zeros((2048, D), np.float32); Gp[:2047] = G * 16.0
    Fp = np.zeros((2048, D), np.float32); Fp[:2047] = G[::-1] * 16.0
    distn_q = {0: Gp, 1: np.zeros((2048, D), np.float32)}
    distf_q = {0: Fp, 1: np.zeros((2048, D), np.float32)}
    distn_q[1][0:1536] = Gp[512:2048]
    distf_q[1][512:2048] = Fp[0:1536]

    in_maps = []
    for c in range(8):
        b, qhc = c // 2, c % 2
        l0 = qhc * L
        m = dict(shared)
        hsTb = hs[b].T
        m["hsT"] = f8(hsTb)
        m["hsTq"] = f8(hsTb[:, l0:l0 + L])
        m["hsres"] = b16(hs[b, l0:l0 + L, :] + inp["bo"].astype(np.float32))
        m["encT"] = f8(enc[b].T)
        # mask enters scores via the x16-scaled k-band (inject divides by 16)
        # and the exp applies a further 0.125: net host scale = 8*16.
        m["mask"] = np.ascontiguousarray(np.broadcast_to(mask[b, 0, 0, :], (S,)) * 128.0)
        m["distn"] = f8(distn_q[qhc].T)
        m["distf"] = f8(distf_q[qhc].T)
        in_maps.append(m)

    nc = _get_nc()
    res = run_bass_kernel_spmd(nc, in_maps, core_ids=list(range(8)))
    LAST_EXEC_NS = res.exec_time_ns
    LAST_MEAN_EXEC_NS = res.mean_exec_time_ns
    LAST_TRACE = res.instructions_and_trace

    out = np.zeros((B, S, DM), np.float32)
    for c in range(8):
        b, qhc = c // 2, c % 2
        out[b, qhc * L:(qhc + 1) * L, :] = res.results[c]["out"]
    return out


# revision 16
# speedup vs baseline: 1.0197x; 1.0197x over previous
"""Trainium2 Bass kernel for JonbertaSelfAttention (B=4,S=1024,DM=1024,H=16,D=64,SE=512,DF=512).

Sharding: 8 cores = (batch b = c//2) x (query-half qh = c%2), ONE NEFF for all
cores (query-half offset folded into per-core data: shifted distance tables and
a pre-sliced hidden_states column block).

v3: fp8e4 DoubleRow everywhere + single merged band/attention pipeline.
 - projections contract it-pairs [128,2,*];
 - QK^T scores contract d=64 via a zero-slot second lane (kT4/qT4/fkT4 carry a
   zeroed e=1 lane, rhs broadcast stride-0);
 - relative-position bias via banded matmuls (x16-scaled fp8 tables), DRAM
   skew round-trip (SWDGE queue), then paired-identity DoubleRow
   transpose/inject (identsplit/identz carry 1/16 to undo the table scale);
 - PV produces ctx in [l, d+1] layout (lhsT=exp pairs) so softmax denominators
   are per-partition scalars; normalized ctx is transposed back by PE;
 - output dense contracts head-pairs from fp8 ctxpk;
 - band evacuation (PSUM->SBUF) split between ACT and DVE; LN applies on Pool.
"""
import os
import numpy as np
import ml_dtypes

BF16 = ml_dtypes.bfloat16
F8 = ml_dtypes.float8_e4m3
B, S, DM, H, D, SE, DF, MAXP = 4, 1024, 1024, 16, 64, 512, 512, 1024
L = 512          # query rows per core
NRT = S // 128   # 8 r-tiles
NLT = L // 128   # 4 l-tiles
NET = SE // 128  # 4 encoder r-tiles
LN_EPS = 1e-12

_CACHE = {}
LAST_EXEC_NS = None
LAST_MEAN_EXEC_NS = None
LAST_TRACE = None

# k-band evacuation engine split (True = ACT, False = DVE), tuned vs trace
K_ON_ACT = (True, False, False, False, True, False, False, False)


def _build():
    import concourse.bass as bass
    import concourse.mybir as mybir
    import concourse.tile as tile
    from concourse import bacc
    from concourse.masks import make_identity
    from contextlib import ExitStack

    dt = mybir.dt
    nc = bacc.Bacc("TRN2", target_bir_lowering=False, debug=False, num_devices=8)

    d_hsT = nc.dram_tensor("hsT", [DM, S], dt.float8e4, kind="ExternalInput")
    d_hsTq = nc.dram_tensor("hsTq", [DM, L], dt.float8e4, kind="ExternalInput")
    d_hsres = nc.dram_tensor("hsres", [L, DM], dt.bfloat16, kind="ExternalInput")
    d_encT = nc.dram_tensor("encT", [DF, SE], dt.float8e4, kind="ExternalInput")
    d_mask = nc.dram_tensor("mask", [S], dt.float32, kind="ExternalInput")
    d_wqT = nc.dram_tensor("wqT", [DM, DM], dt.float8e4, kind="ExternalInput")
    d_wkT = nc.dram_tensor("wkT", [DM, DM], dt.float8e4, kind="ExternalInput")
    d_wvT = nc.dram_tensor("wvT", [DM, DM], dt.float8e4, kind="ExternalInput")
    d_wfkT = nc.dram_tensor("wfkT", [DF, DM], dt.float8e4, kind="ExternalInput")
    d_wfvT = nc.dram_tensor("wfvT", [DF, DM], dt.float8e4, kind="ExternalInput")
    d_woT = nc.dram_tensor("woT", [DM, DM], dt.float8e4, kind="ExternalInput")
    d_bq = nc.dram_tensor("bq", [DM], dt.float32, kind="ExternalInput")
    d_bk = nc.dram_tensor("bk", [DM], dt.float32, kind="ExternalInput")
    d_bfk = nc.dram_tensor("bfk", [DM], dt.float32, kind="ExternalInput")
    d_bv = nc.dram_tensor("bv", [DM], dt.bfloat16, kind="ExternalInput")
    d_bfv = nc.dram_tensor("bfv", [DM], dt.bfloat16, kind="ExternalInput")
    d_lng = nc.dram_tensor("lng", [DM], dt.float32, kind="ExternalInput")
    d_lnb = nc.dram_tensor("lnb", [DM], dt.float32, kind="ExternalInput")
    d_distn = nc.dram_tensor("distn", [D, 2048], dt.float8e4, kind="ExternalInput")
    d_distf = nc.dram_tensor("distf", [D, 2048], dt.float8e4, kind="ExternalInput")
    d_out = nc.dram_tensor("out", [L, DM], dt.float32, kind="ExternalOutput")

    AP = bass.AP
    f32 = dt.float32
    bf16 = dt.bfloat16
    fp8 = dt.float8e4
    AF = mybir.ActivationFunctionType
    DR = mybir.MatmulPerfMode.DoubleRow
    ALU = mybir.AluOpType

    with tile.TileContext(nc) as tc, ExitStack() as top:
        scr = top.enter_context(tc.tile_pool(name="scr", bufs=H, space="DRAM"))
        scr2 = top.enter_context(tc.tile_pool(name="scr2", bufs=H, space="DRAM"))
        pers = top.enter_context(tc.tile_pool(name="pers", bufs=1))
        # e=1 lanes of kT4/qT4/fkT4 are zeros (DoubleRow zero-slot padding)
        kT4 = pers.tile([128, NRT, 2, S], fp8, tag="kT4")
        qT4 = pers.tile([128, NRT, 2, L], fp8, tag="qT4")
        fkT4 = pers.tile([128, NRT, 2, SE], fp8, tag="fkT4")
        v_sb = pers.tile([128, NRT, H, 65], fp8, tag="v_sb")
        fv_sb = pers.tile([128, NET, H, 65], fp8, tag="fv_sb")
        hsres = pers.tile([128, NLT, DM], bf16, tag="hsres")
        ctxpk = pers.tile([64, NRT, 2, L], fp8, tag="ctxpk")
        bv_b = pers.tile([128, DM], bf16, tag="bv_b")
        bfv_b = pers.tile([128, DM], bf16, tag="bfv_b")
        lng_b = pers.tile([128, DM], f32, tag="lng_b")
        lnb_b = pers.tile([128, DM], f32, tag="lnb_b")
        bq_s = pers.tile([128, NRT], f32, tag="bq_s")
        bk_s = pers.tile([128, NRT], f32, tag="bk_s")
        bfk_s = pers.tile([128, NRT], f32, tag="bfk_s")
        msk = pers.tile([128, NRT], f32, tag="msk")
        identb = pers.tile([128, 128], bf16, tag="identb")
        identsplit = pers.tile([128, 2, 256], fp8, tag="identsplit")
        identz = pers.tile([128, 2, 128], fp8, tag="identz")
        eps_t = pers.tile([128, 1], f32, tag="eps_t")
        zero_t = pers.tile([128, 1], f32, tag="zero_t")

        make_identity(nc, identb[:])
        nc.vector.memset(eps_t[:], LN_EPS)
        nc.vector.memset(zero_t[:], 0.0)
        # zero-slot lanes + identity patterns (big memsets on idle Pool engine)
        nc.gpsimd.memset(kT4[:, :, 1, :], 0.0)
        nc.gpsimd.memset(qT4[:, :, 1, :], 0.0)
        nc.gpsimd.memset(fkT4[:, :, 1, :], 0.0)
        nc.gpsimd.memset(identsplit[:], 0.0)
        nc.gpsimd.memset(identz[:, 1, :], 0.0)
        # identsplit: e=0 -> I/16 in cols 0:128, e=1 -> I/16 in cols 128:256
        nc.scalar.activation(out=identsplit[:, 0, 0:128], in_=identb[:],
                             func=AF.Identity, scale=0.0625)
        nc.scalar.activation(out=identsplit[:, 1, 128:256], in_=identb[:],
                             func=AF.Identity, scale=0.0625)
        nc.scalar.activation(out=identz[:, 0, :], in_=identb[:],
                             func=AF.Identity, scale=0.0625)
        nc.sync.dma_start(out=bq_s[:], in_=AP(tensor=d_bq, offset=0, ap=[[1, 128], [128, NRT]]))
        nc.sync.dma_start(out=bk_s[:], in_=AP(tensor=d_bk, offset=0, ap=[[1, 128], [128, NRT]]))
        nc.sync.dma_start(out=bfk_s[:], in_=AP(tensor=d_bfk, offset=0, ap=[[1, 128], [128, NRT]]))
        nc.sync.dma_start(out=msk[:], in_=AP(tensor=d_mask, offset=0, ap=[[1, 128], [128, NRT]]))
        nc.vector.memset(v_sb[:, :, :, 64:65], 1.0)
        nc.vector.memset(fv_sb[:, :, :, 64:65], 1.0)

        pb = top.enter_context(tc.tile_pool(name="pb", bufs=1))
        hsT = pb.tile([128, NRT, S], fp8, tag="hsT")
        hsTq = pb.tile([128, NRT, L], fp8, tag="hsTq")
        encT = pb.tile([128, NET, SE], fp8, tag="encT")
        distn_s = pb.tile([128, 2048], fp8, tag="distn")
        distf_s = pb.tile([128, 2048], fp8, tag="distf")

        nc.sync.dma_start(out=hsTq[:], in_=AP(tensor=d_hsTq, offset=0,
                                              ap=[[L, 128], [128 * L, NRT], [1, L]]))
        nc.sync.dma_start(out=hsT[:], in_=AP(tensor=d_hsT, offset=0,
                                             ap=[[S, 128], [128 * S, NRT], [1, S]]))
        nc.sync.dma_start(out=encT[:], in_=AP(tensor=d_encT, offset=0,
                                              ap=[[SE, 128], [128 * SE, NET], [1, SE]]))
        for half in range(2):
            nc.sync.dma_start(out=distn_s[half * 64:(half + 1) * 64, :],
                              in_=AP(tensor=d_distn, offset=0, ap=[[2048, 64], [1, 2048]]))
            nc.sync.dma_start(out=distf_s[half * 64:(half + 1) * 64, :],
                              in_=AP(tensor=d_distf, offset=0, ap=[[2048, 64], [1, 2048]]))

        cq_dram = {}
        ck_dram = {}

        # ---------- phase B1: all dense projections ----------
        with ExitStack() as phB:
            wp = phB.enter_context(tc.tile_pool(name="wp", bufs=1))
            wq_all = wp.tile([128, NRT, DM], fp8, tag="wq_all")
            wk_all = wp.tile([128, NRT, DM], fp8, tag="wk_all")
            wfk_all = wp.tile([128, NET, DM], fp8, tag="wfk_all")
            wv_k = wp.tile([128, NRT, DM], fp8, tag="wv_k")
            wfv_k = wp.tile([128, NET, DM], fp8, tag="wfv_k")
            pp_proj = phB.enter_context(tc.tile_pool(name="pp_proj", bufs=2, space="PSUM"))

            nc.sync.dma_start(out=wq_all[:], in_=AP(tensor=d_wqT, offset=0,
                                                    ap=[[DM, 128], [128 * DM, NRT], [1, DM]]))
            nc.sync.dma_start(out=wk_all[:], in_=AP(tensor=d_wkT, offset=0,
                                                    ap=[[DM, 128], [128 * DM, NRT], [1, DM]]))
            nc.sync.dma_start(out=wfk_all[:], in_=AP(tensor=d_wfkT, offset=0,
                                                     ap=[[DM, 128], [128 * DM, NET], [1, DM]]))
            nc.sync.dma_start(out=wv_k[:], in_=AP(tensor=d_wvT, offset=0,
                                                  ap=[[DM, 128], [128 * DM, NRT], [1, DM]]))
            nc.sync.dma_start(out=wfv_k[:], in_=AP(tensor=d_wfvT, offset=0,
                                                   ap=[[DM, 128], [128 * DM, NET], [1, DM]]))
            nc.sync.dma_start(out=bv_b[:], in_=AP(tensor=d_bv, offset=0, ap=[[0, 128], [1, DM]]))
            nc.sync.dma_start(out=bfv_b[:], in_=AP(tensor=d_bfv, offset=0, ap=[[0, 128], [1, DM]]))

            for ot in range(NRT):
                # Q projection (local query half)
                ps = pp_proj.tile([128, 512], f32, tag="proj")
                for j in range(NRT // 2):
                    nc.tensor.matmul(ps[:], lhsT=wq_all[:, 2 * j:2 * j + 2, ot * 128:(ot + 1) * 128],
                                     rhs=hsTq[:, 2 * j:2 * j + 2, :],
                                     start=(j == 0), stop=(j == NRT // 2 - 1), perf_mode=DR)
                nc.scalar.activation(out=qT4[:, ot, 0, :], in_=ps[:], func=AF.Identity,
                                     bias=bq_s[:, ot:ot + 1], scale=1.0)
                # K projection (full sequence)
                for sb_i in range(2):
                    ps = pp_proj.tile([128, 512], f32, tag="proj")
                    for j in range(NRT // 2):
                        nc.tensor.matmul(ps[:], lhsT=wk_all[:, 2 * j:2 * j + 2, ot * 128:(ot + 1) * 128],
                                         rhs=hsT[:, 2 * j:2 * j + 2, sb_i * 512:(sb_i + 1) * 512],
                                         start=(j == 0), stop=(j == NRT // 2 - 1), perf_mode=DR)
                    nc.scalar.activation(out=kT4[:, ot, 0, sb_i * 512:(sb_i + 1) * 512], in_=ps[:],
                                         func=AF.Identity, bias=bk_s[:, ot:ot + 1], scale=1.0)
                # FK projection
                ps = pp_proj.tile([128, 512], f32, tag="proj")
                for j in range(NET // 2):
                    nc.tensor.matmul(ps[:], lhsT=wfk_all[:, 2 * j:2 * j + 2, ot * 128:(ot + 1) * 128],
                                     rhs=encT[:, 2 * j:2 * j + 2, :],
                                     start=(j == 0), stop=(j == NET // 2 - 1), perf_mode=DR)
                nc.scalar.activation(out=fkT4[:, ot, 0, :], in_=ps[:], func=AF.Identity,
                                     bias=bfk_s[:, ot:ot + 1], scale=1.0)
                # V (and FV for ot<4) interleave here so ACT/DVE evacs alternate
                st = ot
                for ob in range(2):
                    ps = pp_proj.tile([128, 512], f32, tag="proj")
                    for j in range(NRT // 2):
                        nc.tensor.matmul(ps[:], lhsT=hsT[:, 2 * j:2 * j + 2, st * 128:(st + 1) * 128],
                                         rhs=wv_k[:, 2 * j:2 * j + 2, ob * 512:(ob + 1) * 512],
                                         start=(j == 0), stop=(j == NRT // 2 - 1), perf_mode=DR)
                    nc.vector.tensor_add(
                        out=v_sb[:, st, ob * 8:(ob + 1) * 8, 0:64],
                        in0=ps[:].rearrange("p (h d) -> p h d", d=64),
                        in1=bv_b[:, ob * 512:(ob + 1) * 512].rearrange("p (h d) -> p h d", d=64))
                if ot < NET:
                    for ob in range(2):
                        ps = pp_proj.tile([128, 512], f32, tag="proj")
                        for j in range(NET // 2):
                            nc.tensor.matmul(ps[:], lhsT=encT[:, 2 * j:2 * j + 2, ot * 128:(ot + 1) * 128],
                                             rhs=wfv_k[:, 2 * j:2 * j + 2, ob * 512:(ob + 1) * 512],
                                             start=(j == 0), stop=(j == NET // 2 - 1), perf_mode=DR)
                        nc.vector.tensor_add(
                            out=fv_sb[:, ot, ob * 8:(ob + 1) * 8, 0:64],
                            in0=ps[:].rearrange("p (h d) -> p h d", d=64),
                            in1=bfv_b[:, ob * 512:(ob + 1) * 512].rearrange("p (h d) -> p h d", d=64))

        # ---------- merged bands + attention pipeline ----------
        # PSUM: shared "big" ring (2 x 3 banks) for band AND score tiles,
        # ctxAB (1 bank, self+enc for an lb-pair) and ctxT (1 bank) = 8 banks.
        with ExitStack() as phC:
            bsb = phC.enter_context(tc.tile_pool(name="bsb", bufs=2))
            bsb2 = phC.enter_context(tc.tile_pool(name="bsb2", bufs=2))
            gp = phC.enter_context(tc.tile_pool(name="gp", bufs=2))
            g2 = phC.enter_context(tc.tile_pool(name="g2", bufs=2))
            ep = phC.enter_context(tc.tile_pool(name="ep", bufs=7))
            cp = phC.enter_context(tc.tile_pool(name="cp", bufs=4))
            rp = phC.enter_context(tc.tile_pool(name="rp", bufs=4))
            pp_big = phC.enter_context(tc.tile_pool(name="pp_big", bufs=2, space="PSUM"))
            pp_c = phC.enter_context(tc.tile_pool(name="pp_c", bufs=1, space="PSUM"))

            def emit_bands(h):
                hp = (h % 2) * 64
                ot = h // 2
                cq = scr.tile([NLT * 128, 1152], fp8, tag="cq")
                cq_dram[h] = cq
                qstage = bsb.tile([128, NLT, 1152], fp8, tag="qstage")
                for lt in range(NLT):
                    bm = 896 - lt * 128
                    ps = pp_big.tile([128, 1152], f32, tag="big")
                    lhs = qT4[hp:hp + 64, ot, :, lt * 128:(lt + 1) * 128]
                    for n0, nn in ((0, 512), (512, 512), (1024, 128)):
                        rhs = distf_s[hp:hp + 64, bm + n0:bm + n0 + nn] \
                            .unsqueeze(1).broadcast_to([64, 2, nn])
                        nc.tensor.matmul(ps[:, n0:n0 + nn], lhsT=lhs, rhs=rhs,
                                         start=True, stop=True, perf_mode=DR)
                    # split evacuation at the bank boundary: ACT 512 / DVE 640
                    nc.scalar.copy(out=qstage[:, lt, 0:512], in_=ps[:, 0:512])
                    nc.vector.tensor_copy(out=qstage[:, lt, 512:1152], in_=ps[:, 512:1152])
                nc.gpsimd.dma_start(out=AP(tensor=cq.tensor, offset=cq.offset,
                                           ap=[[1152, 128], [128 * 1152, NLT], [1, 1152]]),
                                    in_=qstage[:])
                ck = scr2.tile([NRT * 128, 640], fp8, tag="ck")
                ck_dram[h] = ck
                kstage = bsb2.tile([128, NRT, 640], fp8, tag="kstage")
                for rt in range(NRT):
                    bt = 896 - 128 * rt
                    ps = pp_big.tile([128, 1152], f32, tag="big")
                    lhs = kT4[hp:hp + 64, ot, :, rt * 128:(rt + 1) * 128]
                    for n0, nn in ((0, 512), (512, 128)):
                        rhs = distn_s[hp:hp + 64, bt + n0:bt + n0 + nn] \
                            .unsqueeze(1).broadcast_to([64, 2, nn])
                        nc.tensor.matmul(ps[:, n0:n0 + nn], lhsT=lhs, rhs=rhs,
                                         start=True, stop=True, perf_mode=DR)
                    if K_ON_ACT[rt]:
                        nc.scalar.activation(out=kstage[:, rt, :], in_=ps[:, 0:640],
                                             func=AF.Identity, bias=msk[:, rt:rt + 1],
                                             scale=1.0)
                    else:
                        nc.vector.scalar_tensor_tensor(
                            out=kstage[:, rt, :], in0=ps[:, 0:640], scalar=msk[:, rt:rt + 1],
                            in1=hsT[:, 0, 0:640], op0=ALU.add, op1=ALU.bypass)
                nc.gpsimd.dma_start(out=AP(tensor=ck.tensor, offset=ck.offset,
                                           ap=[[640, 128], [128 * 640, NRT], [1, 640]]),
                                    in_=kstage[:])

            def attention(h):
                hp = (h % 2) * 64
                ot = h // 2
                b1all = gp.tile([128, NLT, 1024], fp8, tag="b1all")
                src = cq_dram[h]
                nc.sync.dma_start(out=b1all[:], in_=AP(
                    tensor=src.tensor, offset=src.offset + 127,
                    ap=[[1151, 128], [128 * 1152, NLT], [1, 1024]]))
                b2all = g2.tile([128, NRT, 512], fp8, tag="b2all")
                src = ck_dram[h]
                nc.sync.dma_start(out=b2all[:], in_=AP(
                    tensor=src.tensor, offset=src.offset + 127,
                    ap=[[639, 128], [128 * 640, NRT], [1, 512]]))

                def scores_rt(rt):
                    ps = pp_big.tile([128, 512], f32, tag="big")
                    nc.tensor.matmul(
                        ps[:],
                        lhsT=kT4[hp:hp + 64, ot, :, rt * 128:(rt + 1) * 128],
                        rhs=qT4[hp:hp + 64, ot, 0, :].unsqueeze(1).broadcast_to([64, 2, L]),
                        start=True, stop=False, perf_mode=DR, skip_group_check=True)
                    for ltp in (0, 2):
                        nc.tensor.matmul(
                            ps[:, ltp * 128:ltp * 128 + 256],
                            lhsT=b1all[:, ltp:ltp + 2, rt * 128:(rt + 1) * 128],
                            rhs=identsplit[:], start=False, stop=False,
                            perf_mode=DR, skip_group_check=True)
                    nc.tensor.matmul(
                        ps[:], lhsT=identz[:],
                        rhs=b2all[:, rt, :].unsqueeze(1).broadcast_to([128, 2, 512]),
                        start=False, stop=True, perf_mode=DR, skip_group_check=True)
                    return ps

                exs = {}
                for u in range(4):
                    ex4 = ep.tile([128, 2, 512], fp8, tag="ex")
                    for half in range(2):
                        ps = scores_rt(2 * u + half)
                        nc.scalar.activation(out=ex4[:, half, :], in_=ps[:], func=AF.Exp,
                                             bias=zero_t[:], scale=0.125)
                    exs[u] = ex4
                eexs = {}
                for p in range(2):
                    eex4 = ep.tile([128, 2, 512], fp8, tag="ex")
                    for half in range(2):
                        ret = 2 * p + half
                        ps = pp_big.tile([128, 512], f32, tag="big")
                        nc.tensor.matmul(
                            ps[:],
                            lhsT=fkT4[hp:hp + 64, ot, :, ret * 128:(ret + 1) * 128],
                            rhs=qT4[hp:hp + 64, ot, 0, :].unsqueeze(1).broadcast_to([64, 2, L]),
                            start=True, stop=True, perf_mode=DR, skip_group_check=True)
                        nc.scalar.activation(out=eex4[:, half, :], in_=ps[:], func=AF.Exp,
                                             bias=zero_t[:], scale=0.125)
                    eexs[p] = eex4

                # PV in two lb-pair rounds through one ctxAB bank;
                # denominators are per-partition scalars in [l, 65] layout
                ctxT_ps = pp_c.tile([64, 512], f32, tag="ctxT")
                for rnd in range(2):
                    psAB = pp_c.tile([128, 2, 2, 65], f32, tag="ctxAB")
                    for i in range(2):
                        lb = 2 * rnd + i
                        for u in range(4):
                            nc.tensor.matmul(
                                psAB[:, i, 0, :], lhsT=exs[u][:, :, lb * 128:(lb + 1) * 128],
                                rhs=v_sb[:, 2 * u:2 * u + 2, h, :],
                                start=(u == 0), stop=(u == 3),
                                perf_mode=DR, skip_group_check=True)
                        for p in range(2):
                            nc.tensor.matmul(
                                psAB[:, i, 1, :], lhsT=eexs[p][:, :, lb * 128:(lb + 1) * 128],
                                rhs=fv_sb[:, 2 * p:2 * p + 2, h, :],
                                start=(p == 0), stop=(p == 1),
                                perf_mode=DR, skip_group_check=True)
                    rec = rp.tile([128, 2, 2, 1], f32, tag="rec")
                    nc.vector.reciprocal(out=rec[:], in_=psAB[:, :, :, 64:65])
                    for i in range(2):
                        lb = 2 * rnd + i
                        t1 = cp.tile([128, 64], f32, tag="t1")
                        nc.vector.tensor_scalar(out=t1[:], in0=psAB[:, i, 0, 0:64],
                                                scalar1=rec[:, i, 0, :], scalar2=None,
                                                op0=ALU.mult)
                        tnf = cp.tile([128, 64], bf16, tag="tnf")
                        nc.vector.scalar_tensor_tensor(
                            out=tnf[:], in0=psAB[:, i, 1, 0:64], scalar=rec[:, i, 1, :],
                            in1=t1[:], op0=ALU.mult, op1=ALU.add)
                        nc.tensor.matmul(ctxT_ps[:, lb * 128:(lb + 1) * 128],
                                         lhsT=tnf[:], rhs=identb[:],
                                         start=True, stop=True, skip_group_check=True)
                nc.scalar.copy(out=ctxpk[:, ot, h % 2, :], in_=ctxT_ps[:])

            for ot in range(NRT):
                emit_bands(2 * ot)
                emit_bands(2 * ot + 1)
                if ot >= 1:
                    attention(2 * (ot - 1))
                    attention(2 * (ot - 1) + 1)
            attention(2 * (NRT - 1))
            attention(2 * (NRT - 1) + 1)

        # ---------- output dense + residual + LN ----------
        nc.sync.dma_start(out=lng_b[:], in_=AP(tensor=d_lng, offset=0, ap=[[0, 128], [1, DM]]))
        nc.sync.dma_start(out=lnb_b[:], in_=AP(tensor=d_lnb, offset=0, ap=[[0, 128], [1, DM]]))
        nc.sync.dma_start(out=hsres[:], in_=AP(tensor=d_hsres, offset=0,
                                               ap=[[DM, 128], [128 * DM, NLT], [1, DM]]))

        with ExitStack() as phD:
            pd = phD.enter_context(tc.tile_pool(name="pd", bufs=1))
            wo_sb = pd.tile([64, H, DM], fp8, tag="wo_sb")
            yp = phD.enter_context(tc.tile_pool(name="yp", bufs=2))
            op = phD.enter_context(tc.tile_pool(name="op", bufs=2))
            stp = phD.enter_context(tc.tile_pool(name="stp", bufs=2))
            pp_y = phD.enter_context(tc.tile_pool(name="pp_y", bufs=2, space="PSUM"))

            nc.sync.dma_start(out=wo_sb[:], in_=AP(tensor=d_woT, offset=0,
                                                   ap=[[DM, 64], [64 * DM, H], [1, DM]]))
            for st in range(NLT):
                y = yp.tile([128, DM], f32, tag="y")
                for ob in range(2):
                    ps = pp_y.tile([128, 512], f32, tag="py")
                    for hq in range(NRT):
                        nc.tensor.matmul(
                            ps[:], lhsT=ctxpk[:, hq, :, st * 128:(st + 1) * 128],
                            rhs=wo_sb[:, 2 * hq:2 * hq + 2, ob * 512:(ob + 1) * 512],
                            start=(hq == 0), stop=(hq == NRT - 1), perf_mode=DR)
                    nc.vector.tensor_add(out=y[:, ob * 512:(ob + 1) * 512], in0=ps[:],
                                         in1=hsres[:, st, ob * 512:(ob + 1) * 512])
                stats = stp.tile([128, 2, 6], f32, tag="stats")
                nc.vector.bn_stats(out=stats[:, 0, :], in_=y[:, 0:512])
                nc.vector.bn_stats(out=stats[:, 1, :], in_=y[:, 512:1024])
                mv = stp.tile([128, 2], f32, tag="mv")
                nc.vector.bn_aggr(out=mv[:], in_=stats[:])
                sd = stp.tile([128, 1], f32, tag="sd")
                nc.scalar.activation(out=sd[:], in_=mv[:, 1:2], func=AF.Sqrt,
                                     bias=eps_t[:], scale=1.0)
                rstd = stp.tile([128, 1], f32, tag="rstd")
                nc.vector.reciprocal(out=rstd[:], in_=sd[:])
                o1 = op.tile([128, DM], f32, tag="o1")
                nc.vector.tensor_scalar(out=o1[:], in0=y[:], scalar1=mv[:, 0:1], scalar2=rstd[:],
                                        op0=ALU.subtract, op1=ALU.mult)
                o2 = op.tile([128, DM], f32, tag="o2")
                nc.gpsimd.tensor_mul(out=o2[:], in0=o1[:], in1=lng_b[:])
                o3 = op.tile([128, DM], f32, tag="o3")
                nc.gpsimd.tensor_add(out=o3[:], in0=o2[:], in1=lnb_b[:])
                nc.sync.dma_start(out=d_out[st * 128:(st + 1) * 128, :], in_=o3[:])

    nc.finalize()
    return nc


def _get_nc():
    if "nc" not in _CACHE:
        _CACHE["nc"] = _build()
    return _CACHE["nc"]


def kernel(**inputs):
    global LAST_EXEC_NS, LAST_MEAN_EXEC_NS, LAST_TRACE
    from concourse.bass_utils import run_bass_kernel_spmd

    inp = {k: np.asarray(v) for k, v in inputs.items()}
    hs = inp["hidden_states"].astype(np.float32)
    mask = inp["attention_mask"].astype(np.float32)
    enc = inp["encoder_hidden_states"].astype(np.float32)
    G = inp["dist_emb"].astype(np.float32)

    def b16(x):
        return np.ascontiguousarray(x.astype(BF16))

    def f8(x):
        return np.ascontiguousarray(x.astype(F8))

    shared = {
        "wqT": f8(inp["Wq"].T), "wkT": f8(inp["Wk"].T), "wvT": f8(inp["Wv"].T),
        "wfkT": f8(inp["Wfk"].T), "wfvT": f8(inp["Wfv"].T), "woT": f8(inp["Wo"].T),
        "bq": inp["bq"].astype(np.float32), "bk": inp["bk"].astype(np.float32),
        "bfk": inp["bfk"].astype(np.float32), "bv": b16(inp["bv"]), "bfv": b16(inp["bfv"]),
        "lng": inp["ln_g"].astype(np.float32), "lnb": inp["ln_b"].astype(np.float32),
    }
    # Padded tables: G' (natural order) and F' (flipped), plus per-query-half
    # shifted variants so one NEFF (band offsets hardcoded for l0=0) serves
    # both query halves. Tables ship x16 in fp8; the identity injectors carry
    # the 1/16 to undo it.
    Gp = np.zeros((2048, D), np.float32); Gp[:2047] = G * 16.0
    Fp = np.zeros((2048, D), np.float32); Fp[:2047] = G[::-1] * 16.0
    distn_q = {0: Gp, 1: np.zeros((2048, D), np.float32)}
    distf_q = {0: Fp, 1: np.zeros((2048, D), np.float32)}
    distn_q[1][0:1536] = Gp[512:2048]
    distf_q[1][512:2048] = Fp[0:1536]

    in_maps = []
    for c in range(8):
        b, qhc = c // 2, c % 2
        l0 = qhc * L
        m = dict(shared)
        hsTb = hs[b].T
        m["hsT"] = f8(hsTb)
        m["hsTq"] = f8(hsTb[:, l0:l0 + L])
        m["hsres"] = b16(hs[b, l0:l0 + L, :] + inp["bo"].astype(np.float32))
        m["encT"] = f8(enc[b].T)
        # mask enters scores via the x16-scaled k-band (inject divides by 16)
        # and the exp applies a further 0.125: net host scale = 8*16.
        m["mask"] = np.ascontiguousarray(np.broadcast_to(mask[b, 0, 0, :], (S,)) * 128.0)
        m["distn"] = f8(distn_q[qhc].T)
        m["distf"] = f8(distf_q[qhc].T)
        in_maps.append(m)

    nc = _get_nc()
    res = run_bass_kernel_spmd(nc, in_maps, core_ids=list(range(8)))
    LAST_EXEC_NS = res.exec_time_ns
    LAST_MEAN_EXEC_NS = res.mean_exec_time_ns
    LAST_TRACE = res.instructions_and_trace

    out = np.zeros((B, S, DM), np.float32)
    for c in range(8):
        b, qhc = c // 2, c % 2
        out[b, qhc * L:(qhc + 1) * L, :] = res.results[c]["out"]
    return out


# revision 17
# speedup vs baseline: 1.0645x; 1.0439x over previous
"""Trainium2 Bass kernel for JonbertaSelfAttention (B=4,S=1024,DM=1024,H=16,D=64,SE=512,DF=512).

Sharding: 8 cores = (batch b = c//2) x (query-half qh = c%2), ONE NEFF for all
cores (query-half offset folded into per-core data: shifted distance tables and
a pre-sliced hidden_states column block).

v3: fp8e4 DoubleRow everywhere + single merged band/attention pipeline.
 - projections contract it-pairs [128,2,*];
 - QK^T scores contract d=64 via a zero-slot second lane (kT4/qT4/fkT4 carry a
   zeroed e=1 lane, rhs broadcast stride-0);
 - relative-position bias via banded matmuls (x16-scaled fp8 tables), DRAM
   skew round-trip (SWDGE queue), then paired-identity DoubleRow
   transpose/inject (identsplit/identz carry 1/16 to undo the table scale);
 - PV produces ctx in [l, d+1] layout (lhsT=exp pairs) so softmax denominators
   are per-partition scalars; normalized ctx is transposed back by PE;
 - output dense contracts head-pairs from fp8 ctxpk;
 - band evacuation (PSUM->SBUF) split between ACT and DVE; LN applies on Pool.
"""
import os
import numpy as np
import ml_dtypes

BF16 = ml_dtypes.bfloat16
F8 = ml_dtypes.float8_e4m3
B, S, DM, H, D, SE, DF, MAXP = 4, 1024, 1024, 16, 64, 512, 512, 1024
L = 512          # query rows per core
NRT = S // 128   # 8 r-tiles
NLT = L // 128   # 4 l-tiles
NET = SE // 128  # 4 encoder r-tiles
LN_EPS = 1e-12

_CACHE = {}
LAST_EXEC_NS = None
LAST_MEAN_EXEC_NS = None
LAST_TRACE = None

# k-band evacuation engine split (True = ACT, False = DVE), tuned vs trace
K_ON_ACT = (True, False, False, False, True, False, False, False)


def _build():
    import concourse.bass as bass
    import concourse.mybir as mybir
    import concourse.tile as tile
    from concourse import bacc
    from concourse.masks import make_identity
    from contextlib import ExitStack

    dt = mybir.dt
    nc = bacc.Bacc("TRN2", target_bir_lowering=False, debug=False, num_devices=8)

    d_hsT = nc.dram_tensor("hsT", [DM, S], dt.float8e4, kind="ExternalInput")
    d_hsTq = nc.dram_tensor("hsTq", [DM, L], dt.float8e4, kind="ExternalInput")
    d_hsres = nc.dram_tensor("hsres", [L, DM], dt.bfloat16, kind="ExternalInput")
    d_encT = nc.dram_tensor("encT", [DF, SE], dt.float8e4, kind="ExternalInput")
    d_mask = nc.dram_tensor("mask", [S], dt.float32, kind="ExternalInput")
    d_wqT = nc.dram_tensor("wqT", [DM, DM], dt.float8e4, kind="ExternalInput")
    d_wkT = nc.dram_tensor("wkT", [DM, DM], dt.float8e4, kind="ExternalInput")
    d_wvT = nc.dram_tensor("wvT", [DM, DM], dt.float8e4, kind="ExternalInput")
    d_wfkT = nc.dram_tensor("wfkT", [DF, DM], dt.float8e4, kind="ExternalInput")
    d_wfvT = nc.dram_tensor("wfvT", [DF, DM], dt.float8e4, kind="ExternalInput")
    d_woT = nc.dram_tensor("woT", [DM, DM], dt.float8e4, kind="ExternalInput")
    d_bq = nc.dram_tensor("bq", [DM], dt.float32, kind="ExternalInput")
    d_bk = nc.dram_tensor("bk", [DM], dt.float32, kind="ExternalInput")
    d_bfk = nc.dram_tensor("bfk", [DM], dt.float32, kind="ExternalInput")
    d_bv = nc.dram_tensor("bv", [DM], dt.bfloat16, kind="ExternalInput")
    d_bfv = nc.dram_tensor("bfv", [DM], dt.bfloat16, kind="ExternalInput")
    d_lng = nc.dram_tensor("lng", [DM], dt.float32, kind="ExternalInput")
    d_lnb = nc.dram_tensor("lnb", [DM], dt.float32, kind="ExternalInput")
    d_distn = nc.dram_tensor("distn", [D, 2048], dt.float8e4, kind="ExternalInput")
    d_distf = nc.dram_tensor("distf", [D, 2048], dt.float8e4, kind="ExternalInput")
    d_out = nc.dram_tensor("out", [L, DM], dt.float32, kind="ExternalOutput")

    AP = bass.AP
    f32 = dt.float32
    bf16 = dt.bfloat16
    fp8 = dt.float8e4
    AF = mybir.ActivationFunctionType
    DR = mybir.MatmulPerfMode.DoubleRow
    ALU = mybir.AluOpType

    with tile.TileContext(nc) as tc, ExitStack() as top:
        scr = top.enter_context(tc.tile_pool(name="scr", bufs=H, space="DRAM"))
        scr2 = top.enter_context(tc.tile_pool(name="scr2", bufs=H, space="DRAM"))
        pers = top.enter_context(tc.tile_pool(name="pers", bufs=1))
        # e=1 lanes of kT4/qT4/fkT4 are zeros (DoubleRow zero-slot padding)
        kT4 = pers.tile([128, NRT, 2, S], fp8, tag="kT4")
        qT4 = pers.tile([128, NRT, 2, L], fp8, tag="qT4")
        fkT4 = pers.tile([128, NRT, 2, SE], fp8, tag="fkT4")
        v_sb = pers.tile([128, NRT, H, 65], fp8, tag="v_sb")
        fv_sb = pers.tile([128, NET, H, 65], fp8, tag="fv_sb")
        hsres = pers.tile([128, NLT, DM], bf16, tag="hsres")
        ctxpk = pers.tile([64, NRT, 2, L], fp8, tag="ctxpk")
        bv_b = pers.tile([128, DM], bf16, tag="bv_b")
        bfv_b = pers.tile([128, DM], bf16, tag="bfv_b")
        lng_b = pers.tile([128, DM], f32, tag="lng_b")
        lnb_b = pers.tile([128, DM], f32, tag="lnb_b")
        bq_s = pers.tile([128, NRT], f32, tag="bq_s")
        bk_s = pers.tile([128, NRT], f32, tag="bk_s")
        bfk_s = pers.tile([128, NRT], f32, tag="bfk_s")
        msk = pers.tile([128, NRT], f32, tag="msk")
        identb = pers.tile([128, 128], bf16, tag="identb")
        identsplit = pers.tile([128, 2, 256], fp8, tag="identsplit")
        identz = pers.tile([128, 2, 128], fp8, tag="identz")
        eps_t = pers.tile([128, 1], f32, tag="eps_t")
        zero_t = pers.tile([128, 1], f32, tag="zero_t")

        make_identity(nc, identb[:])
        nc.vector.memset(eps_t[:], LN_EPS)
        nc.vector.memset(zero_t[:], 0.0)
        # zero-slot lanes + identity patterns (big memsets on idle Pool engine)
        nc.gpsimd.memset(kT4[:, :, 1, :], 0.0)
        nc.gpsimd.memset(qT4[:, :, 1, :], 0.0)
        nc.gpsimd.memset(fkT4[:, :, 1, :], 0.0)
        nc.gpsimd.memset(identsplit[:], 0.0)
        nc.gpsimd.memset(identz[:, 1, :], 0.0)
        # identsplit: e=0 -> I/16 in cols 0:128, e=1 -> I/16 in cols 128:256
        nc.scalar.activation(out=identsplit[:, 0, 0:128], in_=identb[:],
                             func=AF.Identity, scale=0.0625)
        nc.scalar.activation(out=identsplit[:, 1, 128:256], in_=identb[:],
                             func=AF.Identity, scale=0.0625)
        nc.scalar.activation(out=identz[:, 0, :], in_=identb[:],
                             func=AF.Identity, scale=0.0625)
        nc.sync.dma_start(out=bq_s[:], in_=AP(tensor=d_bq, offset=0, ap=[[1, 128], [128, NRT]]))
        nc.sync.dma_start(out=bk_s[:], in_=AP(tensor=d_bk, offset=0, ap=[[1, 128], [128, NRT]]))
        nc.sync.dma_start(out=bfk_s[:], in_=AP(tensor=d_bfk, offset=0, ap=[[1, 128], [128, NRT]]))
        nc.sync.dma_start(out=msk[:], in_=AP(tensor=d_mask, offset=0, ap=[[1, 128], [128, NRT]]))
        nc.vector.memset(v_sb[:, :, :, 64:65], 1.0)
        nc.vector.memset(fv_sb[:, :, :, 64:65], 1.0)

        pb = top.enter_context(tc.tile_pool(name="pb", bufs=1))
        hsT = pb.tile([128, NRT, S], fp8, tag="hsT")
        hsTq = pb.tile([128, NRT, L], fp8, tag="hsTq")
        encT = pb.tile([128, NET, SE], fp8, tag="encT")
        distn_s = pb.tile([128, 2048], fp8, tag="distn")
        distf_s = pb.tile([128, 2048], fp8, tag="distf")

        nc.sync.dma_start(out=hsTq[:], in_=AP(tensor=d_hsTq, offset=0,
                                              ap=[[L, 128], [128 * L, NRT], [1, L]]))
        nc.sync.dma_start(out=hsT[:], in_=AP(tensor=d_hsT, offset=0,
                                             ap=[[S, 128], [128 * S, NRT], [1, S]]))
        nc.sync.dma_start(out=encT[:], in_=AP(tensor=d_encT, offset=0,
                                              ap=[[SE, 128], [128 * SE, NET], [1, SE]]))
        for half in range(2):
            nc.sync.dma_start(out=distn_s[half * 64:(half + 1) * 64, :],
                              in_=AP(tensor=d_distn, offset=0, ap=[[2048, 64], [1, 2048]]))
            nc.sync.dma_start(out=distf_s[half * 64:(half + 1) * 64, :],
                              in_=AP(tensor=d_distf, offset=0, ap=[[2048, 64], [1, 2048]]))

        cq_dram = {}
        ck_dram = {}

        # ---------- phase B1: all dense projections ----------
        with ExitStack() as phB:
            wp = phB.enter_context(tc.tile_pool(name="wp", bufs=1))
            wq_all = wp.tile([128, NRT, DM], fp8, tag="wq_all")
            wk_all = wp.tile([128, NRT, DM], fp8, tag="wk_all")
            wfk_all = wp.tile([128, NET, DM], fp8, tag="wfk_all")
            wv_k = wp.tile([128, NRT, DM], fp8, tag="wv_k")
            wfv_k = wp.tile([128, NET, DM], fp8, tag="wfv_k")
            pp_proj = phB.enter_context(tc.tile_pool(name="pp_proj", bufs=2, space="PSUM"))

            nc.sync.dma_start(out=wq_all[:], in_=AP(tensor=d_wqT, offset=0,
                                                    ap=[[DM, 128], [128 * DM, NRT], [1, DM]]))
            nc.sync.dma_start(out=wk_all[:], in_=AP(tensor=d_wkT, offset=0,
                                                    ap=[[DM, 128], [128 * DM, NRT], [1, DM]]))
            nc.sync.dma_start(out=wfk_all[:], in_=AP(tensor=d_wfkT, offset=0,
                                                     ap=[[DM, 128], [128 * DM, NET], [1, DM]]))
            nc.sync.dma_start(out=wv_k[:], in_=AP(tensor=d_wvT, offset=0,
                                                  ap=[[DM, 128], [128 * DM, NRT], [1, DM]]))
            nc.sync.dma_start(out=wfv_k[:], in_=AP(tensor=d_wfvT, offset=0,
                                                   ap=[[DM, 128], [128 * DM, NET], [1, DM]]))
            nc.sync.dma_start(out=bv_b[:], in_=AP(tensor=d_bv, offset=0, ap=[[0, 128], [1, DM]]))
            nc.sync.dma_start(out=bfv_b[:], in_=AP(tensor=d_bfv, offset=0, ap=[[0, 128], [1, DM]]))

            for ot in range(NRT):
                # Q projection (local query half)
                ps = pp_proj.tile([128, 512], f32, tag="proj")
                for j in range(NRT // 2):
                    nc.tensor.matmul(ps[:], lhsT=wq_all[:, 2 * j:2 * j + 2, ot * 128:(ot + 1) * 128],
                                     rhs=hsTq[:, 2 * j:2 * j + 2, :],
                                     start=(j == 0), stop=(j == NRT // 2 - 1), perf_mode=DR)
                nc.scalar.activation(out=qT4[:, ot, 0, :], in_=ps[:], func=AF.Identity,
                                     bias=bq_s[:, ot:ot + 1], scale=1.0)
                # K projection (full sequence)
                for sb_i in range(2):
                    ps = pp_proj.tile([128, 512], f32, tag="proj")
                    for j in range(NRT // 2):
                        nc.tensor.matmul(ps[:], lhsT=wk_all[:, 2 * j:2 * j + 2, ot * 128:(ot + 1) * 128],
                                         rhs=hsT[:, 2 * j:2 * j + 2, sb_i * 512:(sb_i + 1) * 512],
                                         start=(j == 0), stop=(j == NRT // 2 - 1), perf_mode=DR)
                    nc.scalar.activation(out=kT4[:, ot, 0, sb_i * 512:(sb_i + 1) * 512], in_=ps[:],
                                         func=AF.Identity, bias=bk_s[:, ot:ot + 1], scale=1.0)
                # FK projection
                ps = pp_proj.tile([128, 512], f32, tag="proj")
                for j in range(NET // 2):
                    nc.tensor.matmul(ps[:], lhsT=wfk_all[:, 2 * j:2 * j + 2, ot * 128:(ot + 1) * 128],
                                     rhs=encT[:, 2 * j:2 * j + 2, :],
                                     start=(j == 0), stop=(j == NET // 2 - 1), perf_mode=DR)
                nc.scalar.activation(out=fkT4[:, ot, 0, :], in_=ps[:], func=AF.Identity,
                                     bias=bfk_s[:, ot:ot + 1], scale=1.0)
                # V (and FV for ot<4) interleave here so ACT/DVE evacs alternate
                st = ot
                for ob in range(2):
                    ps = pp_proj.tile([128, 512], f32, tag="proj")
                    for j in range(NRT // 2):
                        nc.tensor.matmul(ps[:], lhsT=hsT[:, 2 * j:2 * j + 2, st * 128:(st + 1) * 128],
                                         rhs=wv_k[:, 2 * j:2 * j + 2, ob * 512:(ob + 1) * 512],
                                         start=(j == 0), stop=(j == NRT // 2 - 1), perf_mode=DR)
                    nc.vector.tensor_add(
                        out=v_sb[:, st, ob * 8:(ob + 1) * 8, 0:64],
                        in0=ps[:].rearrange("p (h d) -> p h d", d=64),
                        in1=bv_b[:, ob * 512:(ob + 1) * 512].rearrange("p (h d) -> p h d", d=64))
                if ot < NET:
                    for ob in range(2):
                        ps = pp_proj.tile([128, 512], f32, tag="proj")
                        for j in range(NET // 2):
                            nc.tensor.matmul(ps[:], lhsT=encT[:, 2 * j:2 * j + 2, ot * 128:(ot + 1) * 128],
                                             rhs=wfv_k[:, 2 * j:2 * j + 2, ob * 512:(ob + 1) * 512],
                                             start=(j == 0), stop=(j == NET // 2 - 1), perf_mode=DR)
                        nc.vector.tensor_add(
                            out=fv_sb[:, ot, ob * 8:(ob + 1) * 8, 0:64],
                            in0=ps[:].rearrange("p (h d) -> p h d", d=64),
                            in1=bfv_b[:, ob * 512:(ob + 1) * 512].rearrange("p (h d) -> p h d", d=64))

        # ---------- merged bands + attention pipeline ----------
        # PSUM: shared "big" ring (2 x 3 banks) for band AND score tiles,
        # ctxAB (1 bank, self+enc for an lb-pair) and ctxT (1 bank) = 8 banks.
        with ExitStack() as phC:
            bsb = phC.enter_context(tc.tile_pool(name="bsb", bufs=2))
            bsb2 = phC.enter_context(tc.tile_pool(name="bsb2", bufs=2))
            gp = phC.enter_context(tc.tile_pool(name="gp", bufs=2))
            g2 = phC.enter_context(tc.tile_pool(name="g2", bufs=2))
            ep = phC.enter_context(tc.tile_pool(name="ep", bufs=7))
            cp = phC.enter_context(tc.tile_pool(name="cp", bufs=4))
            rp = phC.enter_context(tc.tile_pool(name="rp", bufs=4))
            pp_big = phC.enter_context(tc.tile_pool(name="pp_big", bufs=2, space="PSUM"))
            pp_c = phC.enter_context(tc.tile_pool(name="pp_c", bufs=1, space="PSUM"))

            def emit_bands_gen(h):
                hp = (h % 2) * 64
                ot = h // 2
                cq = scr.tile([NLT * 128, 1152], fp8, tag="cq")
                cq_dram[h] = cq
                qstage = bsb.tile([128, NLT, 1152], fp8, tag="qstage")
                for lt in range(NLT):
                    bm = 896 - lt * 128
                    ps = pp_big.tile([128, 1152], f32, tag="big")
                    lhs = qT4[hp:hp + 64, ot, :, lt * 128:(lt + 1) * 128]
                    for n0, nn in ((0, 512), (512, 512), (1024, 128)):
                        rhs = distf_s[hp:hp + 64, bm + n0:bm + n0 + nn] \
                            .unsqueeze(1).broadcast_to([64, 2, nn])
                        nc.tensor.matmul(ps[:, n0:n0 + nn], lhsT=lhs, rhs=rhs,
                                         start=True, stop=True, perf_mode=DR)
                    # split evacuation at the bank boundary: ACT 512 / DVE 640
                    nc.scalar.copy(out=qstage[:, lt, 0:512], in_=ps[:, 0:512])
                    nc.vector.tensor_copy(out=qstage[:, lt, 512:1152], in_=ps[:, 512:1152])
                    yield
                nc.gpsimd.dma_start(out=AP(tensor=cq.tensor, offset=cq.offset,
                                           ap=[[1152, 128], [128 * 1152, NLT], [1, 1152]]),
                                    in_=qstage[:])
                ck = scr2.tile([NRT * 128, 640], fp8, tag="ck")
                ck_dram[h] = ck
                kstage = bsb2.tile([128, NRT, 640], fp8, tag="kstage")
                for rt in range(NRT):
                    bt = 896 - 128 * rt
                    ps = pp_big.tile([128, 1152], f32, tag="big")
                    lhs = kT4[hp:hp + 64, ot, :, rt * 128:(rt + 1) * 128]
                    for n0, nn in ((0, 512), (512, 128)):
                        rhs = distn_s[hp:hp + 64, bt + n0:bt + n0 + nn] \
                            .unsqueeze(1).broadcast_to([64, 2, nn])
                        nc.tensor.matmul(ps[:, n0:n0 + nn], lhsT=lhs, rhs=rhs,
                                         start=True, stop=True, perf_mode=DR)
                    if K_ON_ACT[rt]:
                        nc.scalar.activation(out=kstage[:, rt, :], in_=ps[:, 0:640],
                                             func=AF.Identity, bias=msk[:, rt:rt + 1],
                                             scale=1.0)
                    else:
                        nc.vector.scalar_tensor_tensor(
                            out=kstage[:, rt, :], in0=ps[:, 0:640], scalar=msk[:, rt:rt + 1],
                            in1=hsT[:, 0, 0:640], op0=ALU.add, op1=ALU.bypass)
                    yield
                nc.gpsimd.dma_start(out=AP(tensor=ck.tensor, offset=ck.offset,
                                           ap=[[640, 128], [128 * 640, NRT], [1, 640]]),
                                    in_=kstage[:])

            def attention_gen(h):
                hp = (h % 2) * 64
                ot = h // 2
                b1all = gp.tile([128, NLT, 1024], fp8, tag="b1all")
                src = cq_dram[h]
                nc.sync.dma_start(out=b1all[:], in_=AP(
                    tensor=src.tensor, offset=src.offset + 127,
                    ap=[[1151, 128], [128 * 1152, NLT], [1, 1024]]))
                b2all = g2.tile([128, NRT, 512], fp8, tag="b2all")
                src = ck_dram[h]
                nc.sync.dma_start(out=b2all[:], in_=AP(
                    tensor=src.tensor, offset=src.offset + 127,
                    ap=[[639, 128], [128 * 640, NRT], [1, 512]]))

                def scores_rt(rt):
                    ps = pp_big.tile([128, 512], f32, tag="big")
                    nc.tensor.matmul(
                        ps[:],
                        lhsT=kT4[hp:hp + 64, ot, :, rt * 128:(rt + 1) * 128],
                        rhs=qT4[hp:hp + 64, ot, 0, :].unsqueeze(1).broadcast_to([64, 2, L]),
                        start=True, stop=False, perf_mode=DR, skip_group_check=True)
                    for ltp in (0, 2):
                        nc.tensor.matmul(
                            ps[:, ltp * 128:ltp * 128 + 256],
                            lhsT=b1all[:, ltp:ltp + 2, rt * 128:(rt + 1) * 128],
                            rhs=identsplit[:], start=False, stop=False,
                            perf_mode=DR, skip_group_check=True)
                    nc.tensor.matmul(
                        ps[:], lhsT=identz[:],
                        rhs=b2all[:, rt, :].unsqueeze(1).broadcast_to([128, 2, 512]),
                        start=False, stop=True, perf_mode=DR, skip_group_check=True)
                    return ps

                exs = {}
                for u in range(4):
                    ex4 = ep.tile([128, 2, 512], fp8, tag="ex")
                    for half in range(2):
                        ps = scores_rt(2 * u + half)
                        nc.scalar.activation(out=ex4[:, half, :], in_=ps[:], func=AF.Exp,
                                             bias=zero_t[:], scale=0.125)
                        yield
                    exs[u] = ex4
                eexs = {}
                for p in range(2):
                    eex4 = ep.tile([128, 2, 512], fp8, tag="ex")
                    for half in range(2):
                        ret = 2 * p + half
                        ps = pp_big.tile([128, 512], f32, tag="big")
                        nc.tensor.matmul(
                            ps[:],
                            lhsT=fkT4[hp:hp + 64, ot, :, ret * 128:(ret + 1) * 128],
                            rhs=qT4[hp:hp + 64, ot, 0, :].unsqueeze(1).broadcast_to([64, 2, L]),
                            start=True, stop=True, perf_mode=DR, skip_group_check=True)
                        nc.scalar.activation(out=eex4[:, half, :], in_=ps[:], func=AF.Exp,
                                             bias=zero_t[:], scale=0.125)
                        yield
                    eexs[p] = eex4

                # PV in two lb-pair rounds through one ctxAB bank;
                # denominators are per-partition scalars in [l, 65] layout
                ctxT_ps = pp_c.tile([64, 512], f32, tag="ctxT")
                for rnd in range(2):
                    psAB = pp_c.tile([128, 2, 2, 65], f32, tag="ctxAB")
                    for i in range(2):
                        lb = 2 * rnd + i
                        for u in range(4):
                            nc.tensor.matmul(
                                psAB[:, i, 0, :], lhsT=exs[u][:, :, lb * 128:(lb + 1) * 128],
                                rhs=v_sb[:, 2 * u:2 * u + 2, h, :],
                                start=(u == 0), stop=(u == 3),
                                perf_mode=DR, skip_group_check=True)
                        for p in range(2):
                            nc.tensor.matmul(
                                psAB[:, i, 1, :], lhsT=eexs[p][:, :, lb * 128:(lb + 1) * 128],
                                rhs=fv_sb[:, 2 * p:2 * p + 2, h, :],
                                start=(p == 0), stop=(p == 1),
                                perf_mode=DR, skip_group_check=True)
                    rec = rp.tile([128, 2, 2, 1], f32, tag="rec")
                    nc.vector.reciprocal(out=rec[:], in_=psAB[:, :, :, 64:65])
                    for i in range(2):
                        lb = 2 * rnd + i
                        t1 = cp.tile([128, 64], f32, tag="t1")
                        nc.vector.tensor_scalar(out=t1[:], in0=psAB[:, i, 0, 0:64],
                                                scalar1=rec[:, i, 0, :], scalar2=None,
                                                op0=ALU.mult)
                        tnf = cp.tile([128, 64], bf16, tag="tnf")
                        nc.vector.scalar_tensor_tensor(
                            out=tnf[:], in0=psAB[:, i, 1, 0:64], scalar=rec[:, i, 1, :],
                            in1=t1[:], op0=ALU.mult, op1=ALU.add)
                        nc.tensor.matmul(ctxT_ps[:, lb * 128:(lb + 1) * 128],
                                         lhsT=tnf[:], rhs=identb[:],
                                         start=True, stop=True, skip_group_check=True)
                nc.scalar.copy(out=ctxpk[:, ot, h % 2, :], in_=ctxT_ps[:])

            def interleave(*gens):
                live = [g for g in gens if g is not None]
                while live:
                    nxt = []
                    for g in live:
                        try:
                            next(g)
                            nxt.append(g)
                        except StopIteration:
                            pass
                    live = nxt

            for t in range(NRT + 1):
                b0 = emit_bands_gen(2 * t) if t < NRT else None
                b1 = emit_bands_gen(2 * t + 1) if t < NRT else None
                a0 = attention_gen(2 * (t - 1)) if t >= 1 else None
                a1 = attention_gen(2 * (t - 1) + 1) if t >= 1 else None
                interleave(b0, a0)
                interleave(b1, a1)

        # ---------- output dense + residual + LN ----------
        nc.sync.dma_start(out=lng_b[:], in_=AP(tensor=d_lng, offset=0, ap=[[0, 128], [1, DM]]))
        nc.sync.dma_start(out=lnb_b[:], in_=AP(tensor=d_lnb, offset=0, ap=[[0, 128], [1, DM]]))
        nc.sync.dma_start(out=hsres[:], in_=AP(tensor=d_hsres, offset=0,
                                               ap=[[DM, 128], [128 * DM, NLT], [1, DM]]))

        with ExitStack() as phD:
            pd = phD.enter_context(tc.tile_pool(name="pd", bufs=1))
            wo_sb = pd.tile([64, H, DM], fp8, tag="wo_sb")
            yp = phD.enter_context(tc.tile_pool(name="yp", bufs=2))
            op = phD.enter_context(tc.tile_pool(name="op", bufs=2))
            stp = phD.enter_context(tc.tile_pool(name="stp", bufs=2))
            pp_y = phD.enter_context(tc.tile_pool(name="pp_y", bufs=2, space="PSUM"))

            nc.sync.dma_start(out=wo_sb[:], in_=AP(tensor=d_woT, offset=0,
                                                   ap=[[DM, 64], [64 * DM, H], [1, DM]]))
            for st in range(NLT):
                y = yp.tile([128, DM], f32, tag="y")
                for ob in range(2):
                    ps = pp_y.tile([128, 512], f32, tag="py")
                    for hq in range(NRT):
                        nc.tensor.matmul(
                            ps[:], lhsT=ctxpk[:, hq, :, st * 128:(st + 1) * 128],
                            rhs=wo_sb[:, 2 * hq:2 * hq + 2, ob * 512:(ob + 1) * 512],
                            start=(hq == 0), stop=(hq == NRT - 1), perf_mode=DR)
                    nc.vector.tensor_add(out=y[:, ob * 512:(ob + 1) * 512], in0=ps[:],
                                         in1=hsres[:, st, ob * 512:(ob + 1) * 512])
                stats = stp.tile([128, 2, 6], f32, tag="stats")
                nc.vector.bn_stats(out=stats[:, 0, :], in_=y[:, 0:512])
                nc.vector.bn_stats(out=stats[:, 1, :], in_=y[:, 512:1024])
                mv = stp.tile([128, 2], f32, tag="mv")
                nc.vector.bn_aggr(out=mv[:], in_=stats[:])
                sd = stp.tile([128, 1], f32, tag="sd")
                nc.scalar.activation(out=sd[:], in_=mv[:, 1:2], func=AF.Sqrt,
                                     bias=eps_t[:], scale=1.0)
                rstd = stp.tile([128, 1], f32, tag="rstd")
                nc.vector.reciprocal(out=rstd[:], in_=sd[:])
                o1 = op.tile([128, DM], f32, tag="o1")
                nc.vector.tensor_scalar(out=o1[:], in0=y[:], scalar1=mv[:, 0:1], scalar2=rstd[:],
                                        op0=ALU.subtract, op1=ALU.mult)
                o2 = op.tile([128, DM], f32, tag="o2")
                nc.gpsimd.tensor_mul(out=o2[:], in0=o1[:], in1=lng_b[:])
                o3 = op.tile([128, DM], f32, tag="o3")
                nc.gpsimd.tensor_add(out=o3[:], in0=o2[:], in1=lnb_b[:])
                nc.sync.dma_start(out=d_out[st * 128:(st + 1) * 128, :], in_=o3[:])

    nc.finalize()
    return nc


def _get_nc():
    if "nc" not in _CACHE:
        _CACHE["nc"] = _build()
    return _CACHE["nc"]


def kernel(**inputs):
    global LAST_EXEC_NS, LAST_MEAN_EXEC_NS, LAST_TRACE
    from concourse.bass_utils import run_bass_kernel_spmd

    inp = {k: np.asarray(v) for k, v in inputs.items()}
    hs = inp["hidden_states"].astype(np.float32)
    mask = inp["attention_mask"].astype(np.float32)
    enc = inp["encoder_hidden_states"].astype(np.float32)
    G = inp["dist_emb"].astype(np.float32)

    def b16(x):
        return np.ascontiguousarray(x.astype(BF16))

    def f8(x):
        return np.ascontiguousarray(x.astype(F8))

    shared = {
        "wqT": f8(inp["Wq"].T), "wkT": f8(inp["Wk"].T), "wvT": f8(inp["Wv"].T),
        "wfkT": f8(inp["Wfk"].T), "wfvT": f8(inp["Wfv"].T), "woT": f8(inp["Wo"].T),
        "bq": inp["bq"].astype(np.float32), "bk": inp["bk"].astype(np.float32),
        "bfk": inp["bfk"].astype(np.float32), "bv": b16(inp["bv"]), "bfv": b16(inp["bfv"]),
        "lng": inp["ln_g"].astype(np.float32), "lnb": inp["ln_b"].astype(np.float32),
    }
    # Padded tables: G' (natural order) and F' (flipped), plus per-query-half
    # shifted variants so one NEFF (band offsets hardcoded for l0=0) serves
    # both query halves. Tables ship x16 in fp8; the identity injectors carry
    # the 1/16 to undo it.
    Gp = np.zeros((2048, D), np.float32); Gp[:2047] = G * 16.0
    Fp = np.zeros((2048, D), np.float32); Fp[:2047] = G[::-1] * 16.0
    distn_q = {0: Gp, 1: np.zeros((2048, D), np.float32)}
    distf_q = {0: Fp, 1: np.zeros((2048, D), np.float32)}
    distn_q[1][0:1536] = Gp[512:2048]
    distf_q[1][512:2048] = Fp[0:1536]

    in_maps = []
    for c in range(8):
        b, qhc = c // 2, c % 2
        l0 = qhc * L
        m = dict(shared)
        hsTb = hs[b].T
        m["hsT"] = f8(hsTb)
        m["hsTq"] = f8(hsTb[:, l0:l0 + L])
        m["hsres"] = b16(hs[b, l0:l0 + L, :] + inp["bo"].astype(np.float32))
        m["encT"] = f8(enc[b].T)
        # mask enters scores via the x16-scaled k-band (inject divides by 16)
        # and the exp applies a further 0.125: net host scale = 8*16.
        m["mask"] = np.ascontiguousarray(np.broadcast_to(mask[b, 0, 0, :], (S,)) * 128.0)
        m["distn"] = f8(distn_q[qhc].T)
        m["distf"] = f8(distf_q[qhc].T)
        in_maps.append(m)

    nc = _get_nc()
    res = run_bass_kernel_spmd(nc, in_maps, core_ids=list(range(8)))
    LAST_EXEC_NS = res.exec_time_ns
    LAST_MEAN_EXEC_NS = res.mean_exec_time_ns
    LAST_TRACE = res.instructions_and_trace

    out = np.zeros((B, S, DM), np.float32)
    for c in range(8):
        b, qhc = c // 2, c % 2
        out[b, qhc * L:(qhc + 1) * L, :] = res.results[c]["out"]
    return out
